# revision 2
# baseline (speedup 1.0000x reference)
"""BayesianNN (attention over memory + 2-pass genome gemv) on 8 Trainium2 cores.

Strategy (memory-bound problem; QKV weights = 709 MB of the 1.45 GB input):
  * Column-shard (tensor-parallel) the three QKV projection matrices across the
    8 cores; each core streams its 3 x [7808, 976] f32 shard (pre-transposed on
    host so the contraction dim lands on SBUF partitions) through a SWDGE
    cast-DMA to fp16 (~line-rate) and matmuls against a resident fp16 x^T with
    f32 PSUM accumulation.
  * Biases are folded into the matmul via an extra contraction row
    (x^T row D == 1.0, W^T row D == bias).
  * Partial attention scores (q k^T over the local j-shard) are AllReduced
    [128,128]; softmax + attn @ v + mean-pool run replicated/sharded on-chip.
  * The [N,N] genome matrices are only ever needed at columns [D:N] (pass 1:
    vals is zero past D; pass 2: only the last 2 outputs matter), so the host
    slices [7816, 130] views - 12 MB instead of 733 MB - row-sharded to match
    each core's pooled shard; the vals @ W gemv becomes a sharded reduce with a
    [130] and a [2] AllReduce.
"""

import numpy as np

D = 7686
M = 128
NH = 128
NO = 2
N = D + NH + NO          # 7816
NCORES = 8
JSH = 976                # padded per-core shard width (16 * 61)
IP = 7808                # padded contraction length (61 * 128); row D is the bias row
NIT = IP // 128          # 61 i-tiles
GCH = [128] * 7 + [80]   # genome/pooled row chunks of the 976-shard
SQRT_D = float(np.sqrt(np.float32(D)))

_COMPILED = None


def _build_program():
    import concourse.bacc as bacc
    import concourse.tile as tile
    import concourse.mybir as mybir
    from concourse import masks

    f32, f16 = mybir.dt.float32, mybir.dt.float16
    AF = mybir.ActivationFunctionType

    nc = bacc.Bacc("TRN2", debug=False, num_devices=NCORES)

    wT = {m: nc.dram_tensor(f"{m}T", [IP, JSH], f32, kind="ExternalInput").ap()
          for m in ("k", "q", "v")}
    xT_d = nc.dram_tensor("xT", [IP, M], f32, kind="ExternalInput").ap()
    g_d = {s: nc.dram_tensor(f"g_{s}", [JSH, NH + NO], f32, kind="ExternalInput").ap()
           for s in ("mu", "sig", "eps")}
    h_d = {s: nc.dram_tensor(f"h_{s}", [NH + NO, NO], f32, kind="ExternalInput").ap()
           for s in ("mu", "sig", "eps")}
    b_d = {s: nc.dram_tensor(f"b_{s}", [NH + NO], f32, kind="ExternalInput").ap()
           for s in ("mu", "sig", "eps")}
    out_d = nc.dram_tensor("out", [NO], f32, kind="ExternalOutput").ap()

    with tile.TileContext(nc) as tc:
        with (
            tc.tile_pool(name="const", bufs=1) as constp,
            tc.tile_pool(name="stream", bufs=4) as streamp,
            tc.tile_pool(name="big", bufs=1) as bigp,
            tc.tile_pool(name="small", bufs=2) as smallp,
            tc.tile_pool(name="gen", bufs=1) as genp,
            tc.tile_pool(name="ps_stream", bufs=2, space="PSUM") as ps_stream,
            tc.tile_pool(name="ps_small", bufs=2, space="PSUM") as ps_small,
            tc.tile_pool(name="dram", bufs=1, space="DRAM") as dramp,
        ):
            # ---- resident constants -------------------------------------
            ident = constp.tile([128, 128], f32)
            masks.make_identity(nc, ident[:])
            inv_m = constp.tile([128, 1], f32)
            nc.vector.memset(inv_m[:], 1.0 / M)

            xT_sb = constp.tile([128, NIT * M], f16)
            nc.gpsimd.dma_start(
                xT_sb[:].rearrange("p (t m) -> p t m", m=M),
                xT_d.rearrange("(t p) m -> p t m", p=128),
            )

            # ---- genome tiles: load + reparameterized sample early ------
            gs = []          # sampled W[:, D:N] row-chunks: [chw, 130] f32
            for ch, chw in enumerate(GCH):
                r0 = ch * 128
                gmu = genp.tile([128, NH + NO], f32, tag=f"gmu{ch}", name=f"gmu{ch}")
                gsg = genp.tile([128, NH + NO], f32, tag=f"gsg{ch}", name=f"gsg{ch}")
                gep = genp.tile([128, NH + NO], f32, tag=f"gep{ch}", name=f"gep{ch}")
                for t, s in ((gmu, "mu"), (gsg, "sig"), (gep, "eps")):
                    nc.gpsimd.dma_start(t[:chw, :], g_d[s][r0:r0 + chw, :])
                nc.vector.tensor_mul(gsg[:chw, :], gsg[:chw, :], gep[:chw, :])
                nc.vector.tensor_add(gsg[:chw, :], gsg[:chw, :], gmu[:chw, :])
                gs.append(gsg)

            # hidden-block columns [D:N, N-2:N] sampled, split [128]+[2] rows
            h2 = []
            for part, (r0, rw) in enumerate(((0, NH), (NH, NO))):
                hmu = genp.tile([128, NO], f32, tag=f"hmu{part}", name=f"hmu{part}")
                hsg = genp.tile([128, NO], f32, tag=f"hsg{part}", name=f"hsg{part}")
                hep = genp.tile([128, NO], f32, tag=f"hep{part}", name=f"hep{part}")
                for t, s in ((hmu, "mu"), (hsg, "sig"), (hep, "eps")):
                    nc.gpsimd.dma_start(t[:rw, :], h_d[s][r0:r0 + rw, :])
                nc.vector.tensor_mul(hsg[:rw, :], hsg[:rw, :], hep[:rw, :])
                nc.vector.tensor_add(hsg[:rw, :], hsg[:rw, :], hmu[:rw, :])
                h2.append(hsg)

            # bias sample as columns ([128,1] + [2,1]) and as a [1,2] row
            b1c = []
            for part, (r0, rw) in enumerate(((0, NH), (NH, NO))):
                bmu = genp.tile([128, 1], f32, tag=f"bmu{part}", name=f"bmu{part}")
                bsg = genp.tile([128, 1], f32, tag=f"bsg{part}", name=f"bsg{part}")
                bep = genp.tile([128, 1], f32, tag=f"bep{part}", name=f"bep{part}")
                for t, s in ((bmu, "mu"), (bsg, "sig"), (bep, "eps")):
                    nc.gpsimd.dma_start(t[:rw, :], b_d[s][r0:r0 + rw])
                nc.vector.tensor_mul(bsg[:rw, :], bsg[:rw, :], bep[:rw, :])
                nc.vector.tensor_add(bsg[:rw, :], bsg[:rw, :], bmu[:rw, :])
                b1c.append(bsg)
            b2r = genp.tile([1, NO], f32)
            bmu2 = genp.tile([1, NO], f32)
            bep2 = genp.tile([1, NO], f32)
            nc.gpsimd.dma_start(b2r[:], b_d["sig"][NH:NH + NO])
            nc.gpsimd.dma_start(bmu2[:], b_d["mu"][NH:NH + NO])
            nc.gpsimd.dma_start(bep2[:], b_d["eps"][NH:NH + NO])
            nc.vector.tensor_mul(b2r[:], b2r[:], bep2[:])
            nc.vector.tensor_add(b2r[:], b2r[:], bmu2[:])

            # DRAM bounce buffers for collectives
            sc_in = dramp.tile([M, M], f32)
            sc_out = dramp.tile([M, M], f32)
            p1_in = dramp.tile([NH + NO], f32)
            p1_out = dramp.tile([NH + NO], f32)
            p2_in = dramp.tile([NO], f32)
            p2_out = dramp.tile([NO], f32)
            groups = [list(range(NCORES))]

            # ---- QKV streaming: k, then q, then v -----------------------
            qkv_sb = {}
            qkvT_sb = {}
            for mat in ("k", "q", "v"):
                ps_a = ps_stream.tile([128, 512], f32, tag="ps_a", name=f"psa_{mat}")
                ps_b = ps_stream.tile([128, JSH - 512], f32, tag="ps_b", name=f"psb_{mat}")
                for it in range(NIT):
                    wt = streamp.tile([128, JSH], f16, tag="wt", name=f"wt_{mat}_{it}")
                    nc.gpsimd.dma_start(wt[:], wT[mat][it * 128:(it + 1) * 128, :])
                    lhsT = xT_sb[:, it * M:(it + 1) * M]
                    nc.tensor.matmul(ps_a[:], lhsT, wt[:, 0:512],
                                     start=(it == 0), stop=(it == NIT - 1))
                    nc.tensor.matmul(ps_b[:], lhsT, wt[:, 512:JSH],
                                     start=(it == 0), stop=(it == NIT - 1))
                sb = bigp.tile([128, JSH], f32, tag=f"{mat}_sb", name=f"{mat}_sb")
                nc.vector.tensor_copy(sb[:, 0:512], ps_a[:])
                nc.vector.tensor_copy(sb[:, 512:JSH], ps_b[:])
                qkv_sb[mat] = sb

                if mat in ("k", "q"):
                    # [m, j] -> [j, m] tiles for the j-contracted scores matmul
                    sbT = bigp.tile([128, 8 * 128], f32, tag=f"{mat}T_sb", name=f"{mat}T_sb")
                    for jt, jw in enumerate(GCH):
                        psT = ps_small.tile([128, 128], f32, tag="psT", name=f"psT_{mat}{jt}")
                        nc.tensor.transpose(
                            psT[:jw, :], sb[:, jt * 128:jt * 128 + jw], ident[:])
                        nc.vector.tensor_copy(
                            sbT[:jw, jt * 128:(jt + 1) * 128], psT[:jw, :])
                    qkvT_sb[mat] = sbT

                if mat == "q":
                    # partial scores over the local j-shard, then AllReduce
                    ps_s = ps_small.tile([128, 128], f32, tag="psT", name="ps_s")
                    for jt, jw in enumerate(GCH):
                        nc.tensor.matmul(
                            ps_s[:],
                            qkvT_sb["q"][:jw, jt * 128:jt * 128 + 128],
                            qkvT_sb["k"][:jw, jt * 128:jt * 128 + 128],
                            start=(jt == 0), stop=(jt == 7))
                    sc_sb = smallp.tile([128, 128], f32)
                    nc.vector.tensor_copy(sc_sb[:], ps_s[:])
                    nc.gpsimd.dma_start(sc_in[:], sc_sb[:])
                    nc.gpsimd.collective_compute(
                        "AllReduce", mybir.AluOpType.add, replica_groups=groups,
                        ins=[sc_in.opt()], outs=[sc_out.opt()])
                    scf = smallp.tile([128, 128], f32)
                    nc.gpsimd.dma_start(scf[:], sc_out[:])
                    # softmax over free axis of s/sqrt(D)
                    mx = smallp.tile([128, 1], f32)
                    nc.vector.tensor_reduce(mx[:], scf[:], axis=mybir.AxisListType.X,
                                            op=mybir.AluOpType.max)
                    nc.vector.tensor_scalar_sub(scf[:], scf[:], mx[:])
                    att = smallp.tile([128, 128], f32)
                    nc.scalar.activation(att[:], scf[:], AF.Exp, scale=1.0 / SQRT_D)
                    ssum = smallp.tile([128, 1], f32)
                    nc.vector.tensor_reduce(ssum[:], att[:], axis=mybir.AxisListType.X,
                                            op=mybir.AluOpType.add)
                    rinv = smallp.tile([128, 1], f32)
                    nc.vector.reciprocal(rinv[:], ssum[:])
                    nc.vector.tensor_scalar_mul(att[:], att[:], rinv[:])
                    psat = ps_small.tile([128, 128], f32, tag="psT", name="psat")
                    nc.tensor.transpose(psat[:], att[:], ident[:])
                    attT = smallp.tile([128, 128], f32)
                    nc.vector.tensor_copy(attT[:], psat[:])

            # ---- ctx = attn @ v ; pooled = mean over m ------------------
            ps_ca = ps_stream.tile([128, 512], f32, tag="ps_a", name="ps_ca")
            ps_cb = ps_stream.tile([128, JSH - 512], f32, tag="ps_b", name="ps_cb")
            nc.tensor.matmul(ps_ca[:], attT[:], qkv_sb["v"][:, 0:512])
            nc.tensor.matmul(ps_cb[:], attT[:], qkv_sb["v"][:, 512:JSH])
            ctx_sb = bigp.tile([128, JSH], f32)
            nc.vector.tensor_copy(ctx_sb[:, 0:512], ps_ca[:])
            nc.vector.tensor_copy(ctx_sb[:, 512:JSH], ps_cb[:])

            ps_p = ps_small.tile([128, 8], f32, tag="ps_gen")
            for ch, chw in enumerate(GCH):
                nc.tensor.matmul(ps_p[:chw, ch:ch + 1],
                                 ctx_sb[:, ch * 128:ch * 128 + chw], inv_m[:])
            pooled = smallp.tile([128, 8], f32)
            nc.vector.memset(pooled[:], 0.0)
            for ch, chw in enumerate(GCH):
                nc.vector.tensor_copy(pooled[:chw, ch:ch + 1], ps_p[:chw, ch:ch + 1])

            # ---- genome pass 1: pre1 = pooled @ W[:D, D:N] (sharded) ----
            ps1 = ps_small.tile([1, NH + NO], f32, tag="ps_gen", name="ps1")
            for ch, chw in enumerate(GCH):
                nc.tensor.matmul(ps1[:], pooled[:chw, ch:ch + 1], gs[ch][:chw, :],
                                 start=(ch == 0), stop=(ch == 7))
            p1row = smallp.tile([1, NH + NO], f32)
            nc.vector.tensor_copy(p1row[:], ps1[:])
            nc.gpsimd.dma_start(p1_in[:], p1row[:])
            nc.gpsimd.collective_compute(
                "AllReduce", mybir.AluOpType.add, replica_groups=groups,
                ins=[p1_in.opt()], outs=[p1_out.opt()])

            # h = tanh(pre1 + b) as columns, scaled 1/8 for the replicated part
            hcols = []
            for part, (r0, rw) in enumerate(((0, NH), (NH, NO))):
                pc = smallp.tile([128, 1], f32, tag=f"pc{part}", name=f"pc{part}")
                nc.gpsimd.dma_start(pc[:rw, :], p1_out[r0:r0 + rw])
                nc.vector.tensor_add(pc[:rw, :], pc[:rw, :], b1c[part][:rw, :])
                nc.scalar.activation(pc[:rw, :], pc[:rw, :], AF.Tanh)
                nc.vector.tensor_scalar_mul(pc[:rw, :], pc[:rw, :], 1.0 / NCORES)
                hcols.append(pc)

            # ---- genome pass 2: out = tanh(vals' @ W[:, N-2:N] + b) -----
            ps2 = ps_small.tile([1, NO], f32, tag="ps_gen", name="ps2")
            for ch, chw in enumerate(GCH):
                nc.tensor.matmul(ps2[:], pooled[:chw, ch:ch + 1],
                                 gs[ch][:chw, NH:NH + NO],
                                 start=(ch == 0), stop=False)
            nc.tensor.matmul(ps2[:], hcols[0][:NH, :], h2[0][:NH, :],
                             start=False, stop=False)
            nc.tensor.matmul(ps2[:], hcols[1][:NO, :], h2[1][:NO, :],
                             start=False, stop=True)
            p2row = smallp.tile([1, NO], f32)
            nc.vector.tensor_copy(p2row[:], ps2[:])
            nc.gpsimd.dma_start(p2_in[:], p2row[:])
            nc.gpsimd.collective_compute(
                "AllReduce", mybir.AluOpType.add, replica_groups=groups,
                ins=[p2_in.opt()], outs=[p2_out.opt()])
            fin = smallp.tile([1, NO], f32)
            nc.gpsimd.dma_start(fin[:], p2_out[:])
            nc.vector.tensor_add(fin[:], fin[:], b2r[:])
            nc.scalar.activation(fin[:], fin[:], AF.Tanh)
            nc.gpsimd.dma_start(out_d[:], fin[:])

    nc.compile()
    return nc


def _shard_inputs(inputs):
    x = np.ascontiguousarray(inputs["x"], dtype=np.float32)
    xT = np.zeros((IP, M), np.float32)
    xT[:D, :] = x.T
    xT[D, :] = 1.0                      # bias row

    widths = [min(961, D - 961 * c) for c in range(NCORES)]
    offs = [961 * c for c in range(NCORES)]

    in_maps = []
    for c in range(NCORES):
        off, w = offs[c], widths[c]
        im = {"xT": xT}
        for mat, Wn, bn in (("q", "Wq", "bq"), ("k", "Wk", "bk"), ("v", "Wv", "bv")):
            Wt = np.zeros((IP, JSH), np.float32)
            Wt[:D, :w] = inputs[Wn][off:off + w, :].T
            Wt[D, :w] = inputs[bn][off:off + w]
            im[f"{mat}T"] = Wt
        for s, name in (("mu", "W_mu"), ("sig", "W_sigma"), ("eps", "eps_w")):
            g = np.zeros((JSH, NH + NO), np.float32)
            g[:w, :] = inputs[name][off:off + w, D:N]
            im[f"g_{s}"] = g
            im[f"h_{s}"] = np.ascontiguousarray(
                inputs[name][D:N, N - NO:N], dtype=np.float32)
        for s, name in (("mu", "bias_mu"), ("sig", "bias_sigma"), ("eps", "eps_b")):
            im[f"b_{s}"] = np.ascontiguousarray(inputs[name][D:N], dtype=np.float32)
        in_maps.append(im)
    return in_maps


def _run(inputs, trace=False):
    global _COMPILED
    from concourse.bass_utils import run_bass_kernel_spmd

    if _COMPILED is None:
        _COMPILED = _build_program()
    in_maps = _shard_inputs(inputs)
    res = run_bass_kernel_spmd(
        _COMPILED, in_maps, core_ids=list(range(NCORES)), trace=trace)
    out = np.asarray(res.results[0]["out"], dtype=np.float32).reshape(NO)
    return out, res


def kernel(**inputs):
    out, _ = _run(inputs, trace=False)
    return out


# revision 4
# speedup vs baseline: 1.1405x; 1.1405x over previous
"""BayesianNN (attention over memory + 2-pass genome gemv) on 8 Trainium2 cores.

Strategy (memory-bound problem; QKV weights = 709 MB of the 1.45 GB input):
  * Column-shard (tensor-parallel) the three QKV projection matrices across the
    8 cores; each core streams its 3 x [7808, 976] f32 shard (pre-transposed on
    host so the contraction dim lands on SBUF partitions) through a SWDGE
    cast-DMA to fp16 (~line-rate) and matmuls against a resident fp16 x^T with
    f32 PSUM accumulation.
  * Biases are folded into the matmul via an extra contraction row
    (x^T row D == 1.0, W^T row D == bias).
  * Partial attention scores (q k^T over the local j-shard) are AllReduced
    [128,128]; softmax + attn @ v + mean-pool run replicated/sharded on-chip.
  * The [N,N] genome matrices are only ever needed at columns [D:N] (pass 1:
    vals is zero past D; pass 2: only the last 2 outputs matter), so the host
    slices [7816, 130] views - 12 MB instead of 733 MB - row-sharded to match
    each core's pooled shard; the vals @ W gemv becomes a sharded reduce with a
    [130] and a [2] AllReduce.
"""

import numpy as np

D = 7686
M = 128
NH = 128
NO = 2
N = D + NH + NO          # 7816
NCORES = 8
JSH = 976                # padded per-core shard width (16 * 61)
IP = 7808                # padded contraction length (61 * 128); row D is the bias row
NIT = IP // 128          # 61 i-tiles
GCH = [128] * 7 + [80]   # genome/pooled row chunks of the 976-shard
SQRT_D = float(np.sqrt(np.float32(D)))

_COMPILED = None


def _build_program():
    import concourse.bacc as bacc
    import concourse.tile as tile
    import concourse.mybir as mybir
    from concourse import masks

    f32, f16 = mybir.dt.float32, mybir.dt.float16
    AF = mybir.ActivationFunctionType

    nc = bacc.Bacc("TRN2", debug=False, num_devices=NCORES)

    wT = {m: nc.dram_tensor(f"{m}T", [IP, JSH], f32, kind="ExternalInput").ap()
          for m in ("k", "q", "v")}
    xT_d = nc.dram_tensor("xT", [IP, M], f32, kind="ExternalInput").ap()
    g_d = {s: nc.dram_tensor(f"g_{s}", [JSH, NH + NO], f32, kind="ExternalInput").ap()
           for s in ("mu", "sig", "eps")}
    h_d = {s: nc.dram_tensor(f"h_{s}", [NH + NO, NO], f32, kind="ExternalInput").ap()
           for s in ("mu", "sig", "eps")}
    b_d = {s: nc.dram_tensor(f"b_{s}", [NH + NO], f32, kind="ExternalInput").ap()
           for s in ("mu", "sig", "eps")}
    out_d = nc.dram_tensor("out", [NO], f32, kind="ExternalOutput").ap()

    with tile.TileContext(nc) as tc:
        with (
            tc.tile_pool(name="const", bufs=1) as constp,
            tc.tile_pool(name="stream", bufs=4) as streamp,
            tc.tile_pool(name="big", bufs=1) as bigp,
            tc.tile_pool(name="small", bufs=2) as smallp,
            tc.tile_pool(name="gen", bufs=1) as genp,
            tc.tile_pool(name="ps_stream", bufs=2, space="PSUM") as ps_stream,
            tc.tile_pool(name="ps_small", bufs=2, space="PSUM") as ps_small,
            tc.tile_pool(name="dram", bufs=1, space="DRAM") as dramp,
        ):
            # ---- resident constants -------------------------------------
            ident = constp.tile([128, 128], f32)
            masks.make_identity(nc, ident[:])
            inv_m = constp.tile([128, 1], f32)
            nc.vector.memset(inv_m[:], 1.0 / M)

            xT_sb = constp.tile([128, NIT * M], f16)
            nc.gpsimd.dma_start(
                xT_sb[:].rearrange("p (t m) -> p t m", m=M),
                xT_d.rearrange("(t p) m -> p t m", p=128),
            )

            # DRAM bounce buffers for collectives
            sc_in = dramp.tile([M, M], f32)
            sc_out = dramp.tile([M, M], f32)
            p1_in = dramp.tile([NH + 2 * NO], f32)
            p1_out = dramp.tile([NH + 2 * NO], f32)
            groups = [list(range(NCORES))]

            # ---- QKV streaming: k, q, v (all stream DMAs before any collective
            # so the GpSimd ring never stalls on the AllReduce) --------------
            qkv_sb = {}
            qkvT_sb = {}
            for mat in ("k", "q", "v"):
                ps_a = ps_stream.tile([128, 512], f32, tag="ps_a", name=f"psa_{mat}")
                ps_b = ps_stream.tile([128, JSH - 512], f32, tag="ps_b", name=f"psb_{mat}")
                for it in range(NIT):
                    wt = streamp.tile([128, JSH], f16, tag="wt", name=f"wt_{mat}_{it}")
                    nc.gpsimd.dma_start(wt[:], wT[mat][it * 128:(it + 1) * 128, :])
                    lhsT = xT_sb[:, it * M:(it + 1) * M]
                    nc.tensor.matmul(ps_a[:], lhsT, wt[:, 0:512],
                                     start=(it == 0), stop=(it == NIT - 1))
                    nc.tensor.matmul(ps_b[:], lhsT, wt[:, 512:JSH],
                                     start=(it == 0), stop=(it == NIT - 1))
                sb = bigp.tile([128, JSH], f32, tag=f"{mat}_sb", name=f"{mat}_sb")
                nc.vector.tensor_copy(sb[:, 0:512], ps_a[:])
                nc.vector.tensor_copy(sb[:, 512:JSH], ps_b[:])
                qkv_sb[mat] = sb

                if mat in ("k", "q"):
                    # [m, j] -> [j, m] tiles for the j-contracted scores matmul
                    sbT = bigp.tile([128, 8 * 128], f32, tag=f"{mat}T_sb", name=f"{mat}T_sb")
                    for jt, jw in enumerate(GCH):
                        psT = ps_small.tile([128, 128], f32, tag="psT", name=f"psT_{mat}{jt}")
                        nc.tensor.transpose(
                            psT[:jw, :], sb[:, jt * 128:jt * 128 + jw], ident[:])
                        nc.vector.tensor_copy(
                            sbT[:jw, jt * 128:(jt + 1) * 128], psT[:jw, :])
                    qkvT_sb[mat] = sbT

                if mat == "q":
                    # partial scores over the local j-shard (PE + DVE only here)
                    ps_s = ps_small.tile([128, 128], f32, tag="psT", name="ps_s")
                    for jt, jw in enumerate(GCH):
                        nc.tensor.matmul(
                            ps_s[:],
                            qkvT_sb["q"][:jw, jt * 128:jt * 128 + 128],
                            qkvT_sb["k"][:jw, jt * 128:jt * 128 + 128],
                            start=(jt == 0), stop=(jt == 7))
                    sc_sb = smallp.tile([128, 128], f32)
                    nc.vector.tensor_copy(sc_sb[:], ps_s[:])
                    nc.sync.dma_start(sc_in[:], sc_sb[:])

            # ---- genome tiles: load + reparameterized sample early ------
            gs = []          # sampled W[:, D:N] row-chunks: [chw, 130] f32
            for ch, chw in enumerate(GCH):
                r0 = ch * 128
                gmu = genp.tile([128, NH + NO], f32, tag=f"gmu{ch}", name=f"gmu{ch}")
                gsg = genp.tile([128, NH + NO], f32, tag=f"gsg{ch}", name=f"gsg{ch}")
                gep = genp.tile([128, NH + NO], f32, tag=f"gep{ch}", name=f"gep{ch}")
                for t, s in ((gmu, "mu"), (gsg, "sig"), (gep, "eps")):
                    nc.gpsimd.dma_start(t[:chw, :], g_d[s][r0:r0 + chw, :])
                nc.vector.tensor_mul(gsg[:chw, :], gsg[:chw, :], gep[:chw, :])
                nc.vector.tensor_add(gsg[:chw, :], gsg[:chw, :], gmu[:chw, :])
                gs.append(gsg)

            # hidden-block columns [D:N, N-2:N] sampled, split [128]+[2] rows
            h2 = []
            for part, (r0, rw) in enumerate(((0, NH), (NH, NO))):
                hmu = genp.tile([128, NO], f32, tag=f"hmu{part}", name=f"hmu{part}")
                hsg = genp.tile([128, NO], f32, tag=f"hsg{part}", name=f"hsg{part}")
                hep = genp.tile([128, NO], f32, tag=f"hep{part}", name=f"hep{part}")
                for t, s in ((hmu, "mu"), (hsg, "sig"), (hep, "eps")):
                    nc.gpsimd.dma_start(t[:rw, :], h_d[s][r0:r0 + rw, :])
                nc.vector.tensor_mul(hsg[:rw, :], hsg[:rw, :], hep[:rw, :])
                nc.vector.tensor_add(hsg[:rw, :], hsg[:rw, :], hmu[:rw, :])
                h2.append(hsg)

            # bias sample as columns ([128,1] + [2,1]) and as a [1,2] row
            b1c = []
            for part, (r0, rw) in enumerate(((0, NH), (NH, NO))):
                bmu = genp.tile([128, 1], f32, tag=f"bmu{part}", name=f"bmu{part}")
                bsg = genp.tile([128, 1], f32, tag=f"bsg{part}", name=f"bsg{part}")
                bep = genp.tile([128, 1], f32, tag=f"bep{part}", name=f"bep{part}")
                for t, s in ((bmu, "mu"), (bsg, "sig"), (bep, "eps")):
                    nc.gpsimd.dma_start(t[:rw, :], b_d[s][r0:r0 + rw])
                nc.vector.tensor_mul(bsg[:rw, :], bsg[:rw, :], bep[:rw, :])
                nc.vector.tensor_add(bsg[:rw, :], bsg[:rw, :], bmu[:rw, :])
                b1c.append(bsg)
            b2r = genp.tile([1, NO], f32)
            bmu2 = genp.tile([1, NO], f32)
            bep2 = genp.tile([1, NO], f32)
            nc.gpsimd.dma_start(b2r[:], b_d["sig"][NH:NH + NO])
            nc.gpsimd.dma_start(bmu2[:], b_d["mu"][NH:NH + NO])
            nc.gpsimd.dma_start(bep2[:], b_d["eps"][NH:NH + NO])
            nc.vector.tensor_mul(b2r[:], b2r[:], bep2[:])
            nc.vector.tensor_add(b2r[:], b2r[:], bmu2[:])


            # ---- scores AllReduce + softmax (overlaps the v stream) ---------
            nc.gpsimd.collective_compute(
                "AllReduce", mybir.AluOpType.add, replica_groups=groups,
                ins=[sc_in.opt()], outs=[sc_out.opt()])
            scf = smallp.tile([128, 128], f32)
            nc.sync.dma_start(scf[:], sc_out[:])
            # softmax over free axis of s/sqrt(D)
            mx = smallp.tile([128, 1], f32)
            nc.vector.tensor_reduce(mx[:], scf[:], axis=mybir.AxisListType.X,
                                    op=mybir.AluOpType.max)
            nc.vector.tensor_scalar_sub(scf[:], scf[:], mx[:])
            att = smallp.tile([128, 128], f32)
            nc.scalar.activation(att[:], scf[:], AF.Exp, scale=1.0 / SQRT_D)
            ssum = smallp.tile([128, 1], f32)
            nc.vector.tensor_reduce(ssum[:], att[:], axis=mybir.AxisListType.X,
                                    op=mybir.AluOpType.add)
            rinv = smallp.tile([128, 1], f32)
            nc.vector.reciprocal(rinv[:], ssum[:])
            nc.vector.tensor_scalar_mul(att[:], att[:], rinv[:])
            psat = ps_small.tile([128, 128], f32, tag="psT", name="psat")
            nc.tensor.transpose(psat[:], att[:], ident[:])
            attT = smallp.tile([128, 128], f32)
            nc.vector.tensor_copy(attT[:], psat[:])

            # ---- ctx = attn @ v ; pooled = mean over m ------------------
            ps_ca = ps_stream.tile([128, 512], f32, tag="ps_a", name="ps_ca")
            ps_cb = ps_stream.tile([128, JSH - 512], f32, tag="ps_b", name="ps_cb")
            nc.tensor.matmul(ps_ca[:], attT[:], qkv_sb["v"][:, 0:512])
            nc.tensor.matmul(ps_cb[:], attT[:], qkv_sb["v"][:, 512:JSH])
            ctx_sb = bigp.tile([128, JSH], f32)
            nc.vector.tensor_copy(ctx_sb[:, 0:512], ps_ca[:])
            nc.vector.tensor_copy(ctx_sb[:, 512:JSH], ps_cb[:])

            ps_p = ps_small.tile([128, 8], f32, tag="ps_gen")
            for ch, chw in enumerate(GCH):
                nc.tensor.matmul(ps_p[:chw, ch:ch + 1],
                                 ctx_sb[:, ch * 128:ch * 128 + chw], inv_m[:])
            pooled = smallp.tile([128, 8], f32)
            nc.vector.memset(pooled[:], 0.0)
            for ch, chw in enumerate(GCH):
                nc.vector.tensor_copy(pooled[:chw, ch:ch + 1], ps_p[:chw, ch:ch + 1])

            # ---- genome pass 1 + pass-2 pooled part, one AllReduce ------
            # pre1 partial [130] and the pass-2 pooled-part partial [2] ride
            # the same AllReduce; the h-part of pass 2 uses the replicated
            # post-AR h, so no second reduction is needed.
            ps1 = ps_small.tile([1, NH + NO], f32, tag="ps_gen", name="ps1")
            for ch, chw in enumerate(GCH):
                nc.tensor.matmul(ps1[:], pooled[:chw, ch:ch + 1], gs[ch][:chw, :],
                                 start=(ch == 0), stop=(ch == 7))
            ps2p = ps_small.tile([1, NO], f32, tag="ps_gen", name="ps2p")
            for ch, chw in enumerate(GCH):
                nc.tensor.matmul(ps2p[:], pooled[:chw, ch:ch + 1],
                                 gs[ch][:chw, NH:NH + NO],
                                 start=(ch == 0), stop=(ch == 7))
            p1row = smallp.tile([1, NH + 2 * NO], f32)
            nc.vector.tensor_copy(p1row[:, 0:NH + NO], ps1[:])
            nc.vector.tensor_copy(p1row[:, NH + NO:NH + 2 * NO], ps2p[:])
            nc.sync.dma_start(p1_in[:], p1row[:])
            nc.gpsimd.collective_compute(
                "AllReduce", mybir.AluOpType.add, replica_groups=groups,
                ins=[p1_in.opt()], outs=[p1_out.opt()])

            # h = tanh(pre1 + b) as columns
            hcols = []
            for part, (r0, rw) in enumerate(((0, NH), (NH, NO))):
                pc = smallp.tile([128, 1], f32, tag=f"pc{part}", name=f"pc{part}")
                nc.sync.dma_start(pc[:rw, :], p1_out[r0:r0 + rw])
                nc.vector.tensor_add(pc[:rw, :], pc[:rw, :], b1c[part][:rw, :])
                nc.scalar.activation(pc[:rw, :], pc[:rw, :], AF.Tanh)
                hcols.append(pc)
            pre2p = smallp.tile([1, NO], f32)
            nc.sync.dma_start(pre2p[:], p1_out[NH + NO:NH + 2 * NO])

            # pass 2 h-part (replicated): psum [1,2] = h @ W[D:N, N-2:N]
            psh = ps_small.tile([1, NO], f32, tag="ps_gen", name="psh")
            nc.tensor.matmul(psh[:], hcols[0][:NH, :], h2[0][:NH, :],
                             start=True, stop=False)
            nc.tensor.matmul(psh[:], hcols[1][:NO, :], h2[1][:NO, :],
                             start=False, stop=True)
            fin = smallp.tile([1, NO], f32)
            nc.vector.tensor_copy(fin[:], psh[:])
            nc.vector.tensor_add(fin[:], fin[:], pre2p[:])
            nc.vector.tensor_add(fin[:], fin[:], b2r[:])
            nc.scalar.activation(fin[:], fin[:], AF.Tanh)
            nc.sync.dma_start(out_d[:], fin[:])

    nc.compile()
    return nc


def _shard_inputs(inputs):
    x = np.ascontiguousarray(inputs["x"], dtype=np.float32)
    xT = np.zeros((IP, M), np.float32)
    xT[:D, :] = x.T
    xT[D, :] = 1.0                      # bias row

    widths = [min(961, D - 961 * c) for c in range(NCORES)]
    offs = [961 * c for c in range(NCORES)]

    in_maps = []
    for c in range(NCORES):
        off, w = offs[c], widths[c]
        im = {"xT": xT}
        for mat, Wn, bn in (("q", "Wq", "bq"), ("k", "Wk", "bk"), ("v", "Wv", "bv")):
            Wt = np.zeros((IP, JSH), np.float32)
            Wt[:D, :w] = inputs[Wn][off:off + w, :].T
            Wt[D, :w] = inputs[bn][off:off + w]
            im[f"{mat}T"] = Wt
        for s, name in (("mu", "W_mu"), ("sig", "W_sigma"), ("eps", "eps_w")):
            g = np.zeros((JSH, NH + NO), np.float32)
            g[:w, :] = inputs[name][off:off + w, D:N]
            im[f"g_{s}"] = g
            im[f"h_{s}"] = np.ascontiguousarray(
                inputs[name][D:N, N - NO:N], dtype=np.float32)
        for s, name in (("mu", "bias_mu"), ("sig", "bias_sigma"), ("eps", "eps_b")):
            im[f"b_{s}"] = np.ascontiguousarray(inputs[name][D:N], dtype=np.float32)
        in_maps.append(im)
    return in_maps


def _run(inputs, trace=False):
    global _COMPILED
    from concourse.bass_utils import run_bass_kernel_spmd

    if _COMPILED is None:
        _COMPILED = _build_program()
    in_maps = _shard_inputs(inputs)
    res = run_bass_kernel_spmd(
        _COMPILED, in_maps, core_ids=list(range(NCORES)), trace=trace)
    out = np.asarray(res.results[0]["out"], dtype=np.float32).reshape(NO)
    return out, res


def kernel(**inputs):
    out, _ = _run(inputs, trace=False)
    return out


# revision 9
# speedup vs baseline: 1.1925x; 1.0456x over previous
"""BayesianNN (attention over memory + 2-pass genome gemv) on 8 Trainium2 cores.

Strategy (memory-bound problem; QKV weights = 709 MB of the 1.45 GB input):
  * Column-shard (tensor-parallel) the three QKV projection matrices across the
    8 cores; each core streams its 3 x [7808, 976] f32 shard (pre-transposed on
    host so the contraction dim lands on SBUF partitions) through a SWDGE
    cast-DMA to fp16 (~line-rate) and matmuls against a resident fp16 x^T with
    f32 PSUM accumulation.
  * Biases are folded into the matmul via an extra contraction row
    (x^T row D == 1.0, W^T row D == bias).
  * Partial attention scores (q k^T over the local j-shard) are AllReduced
    [128,128]; softmax + attn @ v + mean-pool run replicated/sharded on-chip.
  * The [N,N] genome matrices are only ever needed at columns [D:N] (pass 1:
    vals is zero past D; pass 2: only the last 2 outputs matter), so the host
    slices [7816, 130] views - 12 MB instead of 733 MB - row-sharded to match
    each core's pooled shard; the vals @ W gemv becomes a sharded reduce with a
    [130] and a [2] AllReduce.
"""

import numpy as np

D = 7686
M = 128
NH = 128
NO = 2
N = D + NH + NO          # 7816
NCORES = 8
JSH = 976                # padded per-core shard width (16 * 61)
IP = 7808                # padded contraction length (61 * 128); row D is the bias row
NIT = IP // 128          # 61 i-tiles
GCH = [128] * 7 + [80]   # genome/pooled row chunks of the 976-shard
SQRT_D = float(np.sqrt(np.float32(D)))

_COMPILED = None


def _build_program():
    import concourse.bacc as bacc
    import concourse.tile as tile
    import concourse.mybir as mybir
    from concourse import masks

    f32, f16 = mybir.dt.float32, mybir.dt.float16
    AF = mybir.ActivationFunctionType

    nc = bacc.Bacc("TRN2", debug=False, num_devices=NCORES)

    wT = {m: nc.dram_tensor(f"{m}T", [IP, JSH], f32, kind="ExternalInput").ap()
          for m in ("k", "q", "v")}
    xT_d = nc.dram_tensor("xT", [IP, M], f32, kind="ExternalInput").ap()
    g_d = {s: nc.dram_tensor(f"g_{s}", [JSH, NH + NO], f32, kind="ExternalInput").ap()
           for s in ("mu", "sig", "eps")}
    h_d = {s: nc.dram_tensor(f"h_{s}", [NH + NO, NO], f32, kind="ExternalInput").ap()
           for s in ("mu", "sig", "eps")}
    b_d = {s: nc.dram_tensor(f"b_{s}", [NH + NO], f32, kind="ExternalInput").ap()
           for s in ("mu", "sig", "eps")}
    out_d = nc.dram_tensor("out", [NO], f32, kind="ExternalOutput").ap()

    with tile.TileContext(nc) as tc:
        with (
            tc.tile_pool(name="const", bufs=1) as constp,
            tc.tile_pool(name="stream", bufs=8) as streamp,
            tc.tile_pool(name="big", bufs=1) as bigp,
            tc.tile_pool(name="small", bufs=2) as smallp,
            tc.tile_pool(name="gen", bufs=1) as genp,
            tc.tile_pool(name="ps_stream", bufs=2, space="PSUM") as ps_stream,
            tc.tile_pool(name="ps_small", bufs=2, space="PSUM") as ps_small,
            tc.tile_pool(name="dram", bufs=1, space="DRAM") as dramp,
        ):
            # ---- resident constants -------------------------------------
            ident = constp.tile([128, 128], f32)
            masks.make_identity(nc, ident[:])
            inv_m = constp.tile([128, 1], f32)
            nc.vector.memset(inv_m[:], 1.0 / M)

            xT_sb = constp.tile([128, NIT * M], f16)
            xT3 = xT_d.rearrange("(t p) m -> p t m", p=128)
            xs3 = xT_sb[:].rearrange("p (t m) -> p t m", m=M)
            for c0 in range(0, NIT, 8):
                cw = min(8, NIT - c0)
                nc.gpsimd.dma_start(xs3[:, c0:c0 + cw, :], xT3[:, c0:c0 + cw, :])

            # DRAM bounce buffers for collectives
            sc_in = dramp.tile([M, M], f32)
            sc_out = dramp.tile([M, M], f32)
            p1_in = dramp.tile([NH + 2 * NO], f32)
            p1_out = dramp.tile([NH + 2 * NO], f32)
            groups = [list(range(NCORES))]

            # ---- genome tiles (emitted after the k-stream so they do not
            # delay the start of the weight stream) ---------------------------
            gs = []          # sampled W[:D, D:N] row-chunks: [chw, 130] f32
            h2 = []          # sampled W[D:N, N-2:N] split [128]+[2] rows
            b1c = []         # sampled bias[D:N] as columns
            b2box = []       # sampled bias[N-2:N] as a [1,2] row

            def emit_genome():
                for ch, chw in enumerate(GCH):
                    r0 = ch * 128
                    gmu = genp.tile([128, NH + NO], f32, tag=f"gmu{ch}", name=f"gmu{ch}")
                    gsg = genp.tile([128, NH + NO], f32, tag=f"gsg{ch}", name=f"gsg{ch}")
                    gep = genp.tile([128, NH + NO], f32, tag=f"gep{ch}", name=f"gep{ch}")
                    for t, s in ((gmu, "mu"), (gsg, "sig"), (gep, "eps")):
                        nc.gpsimd.dma_start(t[:chw, :], g_d[s][r0:r0 + chw, :])
                    nc.vector.tensor_mul(gsg[:chw, :], gsg[:chw, :], gep[:chw, :])
                    nc.vector.tensor_add(gsg[:chw, :], gsg[:chw, :], gmu[:chw, :])
                    gs.append(gsg)

                for part, (r0, rw) in enumerate(((0, NH), (NH, NO))):
                    hmu = genp.tile([128, NO], f32, tag=f"hmu{part}", name=f"hmu{part}")
                    hsg = genp.tile([128, NO], f32, tag=f"hsg{part}", name=f"hsg{part}")
                    hep = genp.tile([128, NO], f32, tag=f"hep{part}", name=f"hep{part}")
                    for t, s in ((hmu, "mu"), (hsg, "sig"), (hep, "eps")):
                        nc.gpsimd.dma_start(t[:rw, :], h_d[s][r0:r0 + rw, :])
                    nc.vector.tensor_mul(hsg[:rw, :], hsg[:rw, :], hep[:rw, :])
                    nc.vector.tensor_add(hsg[:rw, :], hsg[:rw, :], hmu[:rw, :])
                    h2.append(hsg)

                for part, (r0, rw) in enumerate(((0, NH), (NH, NO))):
                    bmu = genp.tile([128, 1], f32, tag=f"bmu{part}", name=f"bmu{part}")
                    bsg = genp.tile([128, 1], f32, tag=f"bsg{part}", name=f"bsg{part}")
                    bep = genp.tile([128, 1], f32, tag=f"bep{part}", name=f"bep{part}")
                    for t, s in ((bmu, "mu"), (bsg, "sig"), (bep, "eps")):
                        nc.gpsimd.dma_start(t[:rw, :], b_d[s][r0:r0 + rw])
                    nc.vector.tensor_mul(bsg[:rw, :], bsg[:rw, :], bep[:rw, :])
                    nc.vector.tensor_add(bsg[:rw, :], bsg[:rw, :], bmu[:rw, :])
                    b1c.append(bsg)
                b2r = genp.tile([1, NO], f32)
                bmu2 = genp.tile([1, NO], f32)
                bep2 = genp.tile([1, NO], f32)
                nc.gpsimd.dma_start(b2r[:], b_d["sig"][NH:NH + NO])
                nc.gpsimd.dma_start(bmu2[:], b_d["mu"][NH:NH + NO])
                nc.gpsimd.dma_start(bep2[:], b_d["eps"][NH:NH + NO])
                nc.vector.tensor_mul(b2r[:], b2r[:], bep2[:])
                nc.vector.tensor_add(b2r[:], b2r[:], bmu2[:])
                b2box.append(b2r)

            # ---- QKV streaming: k, q, v (all stream DMAs before any collective
            # so the GpSimd ring never stalls on the AllReduce) --------------
            qkv_sb = {}
            qkvT_sb = {}
            for mat in ("k", "q", "v"):
                ps_a = ps_stream.tile([128, 512], f32, tag="ps_a", name=f"psa_{mat}")
                ps_b = ps_stream.tile([128, JSH - 512], f32, tag="ps_b", name=f"psb_{mat}")
                for it in range(NIT):
                    wt = streamp.tile([128, JSH], f16, tag="wt", name=f"wt_{mat}_{it}")
                    nc.gpsimd.dma_start(wt[:], wT[mat][it * 128:(it + 1) * 128, :])
                    lhsT = xT_sb[:, it * M:(it + 1) * M]
                    nc.tensor.matmul(ps_a[:], lhsT, wt[:, 0:512],
                                     start=(it == 0), stop=(it == NIT - 1))
                    nc.tensor.matmul(ps_b[:], lhsT, wt[:, 512:JSH],
                                     start=(it == 0), stop=(it == NIT - 1))
                sb = bigp.tile([128, JSH], f32, tag=f"{mat}_sb", name=f"{mat}_sb")
                nc.vector.tensor_copy(sb[:, 0:512], ps_a[:])
                nc.vector.tensor_copy(sb[:, 512:JSH], ps_b[:])
                qkv_sb[mat] = sb

                if mat == "k":
                    emit_genome()

                if mat in ("k", "q"):
                    # [m, j] -> [j, m] tiles for the j-contracted scores matmul
                    sbT = bigp.tile([128, 8 * 128], f32, tag=f"{mat}T_sb", name=f"{mat}T_sb")
                    for jt, jw in enumerate(GCH):
                        psT = ps_small.tile([128, 128], f32, tag="psT", name=f"psT_{mat}{jt}")
                        nc.tensor.transpose(
                            psT[:jw, :], sb[:, jt * 128:jt * 128 + jw], ident[:])
                        nc.vector.tensor_copy(
                            sbT[:jw, jt * 128:(jt + 1) * 128], psT[:jw, :])
                    qkvT_sb[mat] = sbT

                if mat == "q":
                    # partial scores over the local j-shard (PE + DVE only here)
                    ps_s = ps_small.tile([128, 128], f32, tag="psT", name="ps_s")
                    for jt, jw in enumerate(GCH):
                        nc.tensor.matmul(
                            ps_s[:],
                            qkvT_sb["q"][:jw, jt * 128:jt * 128 + 128],
                            qkvT_sb["k"][:jw, jt * 128:jt * 128 + 128],
                            start=(jt == 0), stop=(jt == 7))
                    sc_sb = smallp.tile([128, 128], f32)
                    nc.vector.tensor_copy(sc_sb[:], ps_s[:])
                    nc.sync.dma_start(sc_in[:], sc_sb[:])

            # ---- scores AllReduce + softmax (overlaps the v stream) ---------
            nc.gpsimd.collective_compute(
                "AllReduce", mybir.AluOpType.add, replica_groups=groups,
                ins=[sc_in.opt()], outs=[sc_out.opt()])
            scf = smallp.tile([128, 128], f32)
            nc.sync.dma_start(scf[:], sc_out[:])
            # softmax over free axis of s/sqrt(D)
            mx = smallp.tile([128, 1], f32)
            nc.vector.tensor_reduce(mx[:], scf[:], axis=mybir.AxisListType.X,
                                    op=mybir.AluOpType.max)
            nc.vector.tensor_scalar_sub(scf[:], scf[:], mx[:])
            att = smallp.tile([128, 128], f32)
            nc.scalar.activation(att[:], scf[:], AF.Exp, scale=1.0 / SQRT_D)
            ssum = smallp.tile([128, 1], f32)
            nc.vector.tensor_reduce(ssum[:], att[:], axis=mybir.AxisListType.X,
                                    op=mybir.AluOpType.add)
            rinv = smallp.tile([128, 1], f32)
            nc.vector.reciprocal(rinv[:], ssum[:])
            nc.vector.tensor_scalar_mul(att[:], att[:], rinv[:])
            psat = ps_small.tile([128, 128], f32, tag="psT", name="psat")
            nc.tensor.transpose(psat[:], att[:], ident[:])
            attT = smallp.tile([128, 128], f32)
            nc.vector.tensor_copy(attT[:], psat[:])

            # ---- ctx = attn @ v ; pooled = mean over m ------------------
            ps_ca = ps_stream.tile([128, 512], f32, tag="ps_a", name="ps_ca")
            ps_cb = ps_stream.tile([128, JSH - 512], f32, tag="ps_b", name="ps_cb")
            nc.tensor.matmul(ps_ca[:], attT[:], qkv_sb["v"][:, 0:512])
            nc.tensor.matmul(ps_cb[:], attT[:], qkv_sb["v"][:, 512:JSH])
            ctx_sb = bigp.tile([128, JSH], f32)
            nc.vector.tensor_copy(ctx_sb[:, 0:512], ps_ca[:])
            nc.vector.tensor_copy(ctx_sb[:, 512:JSH], ps_cb[:])

            ps_p = ps_small.tile([128, 8], f32, tag="ps_gen")
            for ch, chw in enumerate(GCH):
                nc.tensor.matmul(ps_p[:chw, ch:ch + 1],
                                 ctx_sb[:, ch * 128:ch * 128 + chw], inv_m[:])
            pooled = smallp.tile([128, 8], f32)
            nc.vector.memset(pooled[:], 0.0)
            for ch, chw in enumerate(GCH):
                nc.vector.tensor_copy(pooled[:chw, ch:ch + 1], ps_p[:chw, ch:ch + 1])

            # ---- genome pass 1 + pass-2 pooled part, one AllReduce ------
            # pre1 partial [130] and the pass-2 pooled-part partial [2] ride
            # the same AllReduce; the h-part of pass 2 uses the replicated
            # post-AR h, so no second reduction is needed.
            ps1 = ps_small.tile([1, NH + NO], f32, tag="ps_gen", name="ps1")
            for ch, chw in enumerate(GCH):
                nc.tensor.matmul(ps1[:], pooled[:chw, ch:ch + 1], gs[ch][:chw, :],
                                 start=(ch == 0), stop=(ch == 7))
            ps2p = ps_small.tile([1, NO], f32, tag="ps_gen", name="ps2p")
            for ch, chw in enumerate(GCH):
                nc.tensor.matmul(ps2p[:], pooled[:chw, ch:ch + 1],
                                 gs[ch][:chw, NH:NH + NO],
                                 start=(ch == 0), stop=(ch == 7))
            p1row = smallp.tile([1, NH + 2 * NO], f32)
            nc.vector.tensor_copy(p1row[:, 0:NH + NO], ps1[:])
            nc.vector.tensor_copy(p1row[:, NH + NO:NH + 2 * NO], ps2p[:])
            nc.sync.dma_start(p1_in[:], p1row[:])
            nc.gpsimd.collective_compute(
                "AllReduce", mybir.AluOpType.add, replica_groups=groups,
                ins=[p1_in.opt()], outs=[p1_out.opt()])

            # h = tanh(pre1 + b) as columns
            hcols = []
            for part, (r0, rw) in enumerate(((0, NH), (NH, NO))):
                pc = smallp.tile([128, 1], f32, tag=f"pc{part}", name=f"pc{part}")
                nc.sync.dma_start(pc[:rw, :], p1_out[r0:r0 + rw])
                nc.vector.tensor_add(pc[:rw, :], pc[:rw, :], b1c[part][:rw, :])
                nc.scalar.activation(pc[:rw, :], pc[:rw, :], AF.Tanh)
                hcols.append(pc)
            pre2p = smallp.tile([1, NO], f32)
            nc.sync.dma_start(pre2p[:], p1_out[NH + NO:NH + 2 * NO])

            # pass 2 h-part (replicated): psum [1,2] = h @ W[D:N, N-2:N]
            psh = ps_small.tile([1, NO], f32, tag="ps_gen", name="psh")
            nc.tensor.matmul(psh[:], hcols[0][:NH, :], h2[0][:NH, :],
                             start=True, stop=False)
            nc.tensor.matmul(psh[:], hcols[1][:NO, :], h2[1][:NO, :],
                             start=False, stop=True)
            fin = smallp.tile([1, NO], f32)
            nc.vector.tensor_copy(fin[:], psh[:])
            nc.vector.tensor_add(fin[:], fin[:], pre2p[:])
            nc.vector.tensor_add(fin[:], fin[:], b2box[0][:])
            nc.scalar.activation(fin[:], fin[:], AF.Tanh)
            nc.sync.dma_start(out_d[:], fin[:])

    nc.compile()
    return nc


def _shard_inputs(inputs):
    x = np.ascontiguousarray(inputs["x"], dtype=np.float32)
    xT = np.zeros((IP, M), np.float32)
    xT[:D, :] = x.T
    xT[D, :] = 1.0                      # bias row

    widths = [min(961, D - 961 * c) for c in range(NCORES)]
    offs = [961 * c for c in range(NCORES)]

    in_maps = []
    for c in range(NCORES):
        off, w = offs[c], widths[c]
        im = {"xT": xT}
        for mat, Wn, bn in (("q", "Wq", "bq"), ("k", "Wk", "bk"), ("v", "Wv", "bv")):
            Wt = np.zeros((IP, JSH), np.float32)
            Wt[:D, :w] = inputs[Wn][off:off + w, :].T
            Wt[D, :w] = inputs[bn][off:off + w]
            im[f"{mat}T"] = Wt
        for s, name in (("mu", "W_mu"), ("sig", "W_sigma"), ("eps", "eps_w")):
            g = np.zeros((JSH, NH + NO), np.float32)
            g[:w, :] = inputs[name][off:off + w, D:N]
            im[f"g_{s}"] = g
            im[f"h_{s}"] = np.ascontiguousarray(
                inputs[name][D:N, N - NO:N], dtype=np.float32)
        for s, name in (("mu", "bias_mu"), ("sig", "bias_sigma"), ("eps", "eps_b")):
            im[f"b_{s}"] = np.ascontiguousarray(inputs[name][D:N], dtype=np.float32)
        in_maps.append(im)
    return in_maps


def _run(inputs, trace=False):
    global _COMPILED
    from concourse.bass_utils import run_bass_kernel_spmd

    if _COMPILED is None:
        _COMPILED = _build_program()
    in_maps = _shard_inputs(inputs)
    res = run_bass_kernel_spmd(
        _COMPILED, in_maps, core_ids=list(range(NCORES)), trace=trace)
    out = np.asarray(res.results[0]["out"], dtype=np.float32).reshape(NO)
    return out, res


def kernel(**inputs):
    out, _ = _run(inputs, trace=False)
    return out
